# revision 14
# baseline (speedup 1.0000x reference)
"""GCN feature extractor on 8 Trainium2 NeuronCores.

Distribution: nodes block-sharded over 8 cores (12500 each, padded to 12544),
degree-sorted within core so 128-node destination tiles have uniform neighbor
counts. Per GCN layer each core computes dinv*(h@W) for its slice, writes it
contiguously to DRAM, and an AllGather with a Shared-scratchpad output
materializes one chip-wide node table. Neighbor features are fetched with
large multi-thousand-row dma_gather instructions (SWDGE, 4 rotating queues,
single_packet off). int16 gather indices only span 64K rows, so sources are
covered by 3 overlapping 64K-row windows with per-tile flex balancing; each
destination tile's slots are split per window and summed on the vector
engine. BatchNorm stats combine via a tiny AllReduce. Pooling runs on a
rank->id permute-gather of the local slice, graph-aligned 128-row chunks,
per-chunk PE transposes + DVE reduces, route matmuls into [64, G], and
AllReduce(add)/AllReduce(max) across cores; every core emits the full output.
"""

import numpy as np

N = 100000
E = 1600000
D = 64
G = 256
NC = 8
NPC_RAW = 12500
NPC = 12544          # 98 tiles of 128
NT = NPC // 128      # 98
TR = NC * NPC        # 100352 table rows
EPS = 1e-5
GPC = G // NC        # 32 graphs per core
HNPC = NPC // 2      # 6272 columns per half of feature-major layout
HT = NT // 2
DEG_INF = np.float32(1e38)
NEG = np.float32(-1e38)

# overlapping signed 64K-row windows: window w covers [LO[w], LO[w]+65536);
# idx = row - (LO[w]+32768) in [-32768, 32768). Every 8th slab column is a
# pad column pointing at a zero row ABOVE the base, so each 8-column
# single-packet sub-gather ends on a non-negative index (trailing-negative
# trim would otherwise break SWDGE descriptor accounting and hang).
WIN_LO = [0, 17408, 34816]
WIN_BASE = [lo + 32768 for lo in WIN_LO]
NW = 3
WSPAN = 65536


def _padpos(k):
    """position of real column k in the slab layout (trailing pad only)."""
    return k


def _padlen(K):
    """slab column count for K real columns (one trailing pad col)."""
    return K + 1 if K else 0
CHUNK_COLS = 96      # target real columns per gather chunk
NG_SLOT = 36         # pooling graph slots per core (own 32 + boundary margin)


def _idx_to_sbuf(idx_flat):
    """[n] -> [128, ceil(n/16)] int16 (pos m -> row m%16, col m//16, x8)."""
    n = len(idx_flat)
    F = (n + 15) // 16
    pad = np.zeros(F * 16, dtype=np.int64)
    pad[:n] = idx_flat
    arr = pad.reshape(F, 16).T.astype(np.int16)
    return np.tile(arr, (8, 1))


def _preprocess(edge_index, batch):
    src = np.asarray(edge_index[0], dtype=np.int64)
    dst = np.asarray(edge_index[1], dtype=np.int64)
    batch = np.asarray(batch, dtype=np.int64)

    core = np.arange(N, dtype=np.int64) // NPC_RAW
    indeg = np.bincount(dst, minlength=N).astype(np.int64)

    # rank within core by descending degree; node -> (p, t); table row
    order = np.lexsort((np.arange(N), -indeg, core))
    core_sorted = core[order]
    block_start = np.searchsorted(core_sorted, np.arange(NC))
    rank = np.arange(N, dtype=np.int64) - block_start[core_sorted]
    rho = np.empty(N, dtype=np.int64)
    rho[order] = rank                      # rho within core
    p_of = rho % 128
    t_of = rho // 128
    tid = core * NPC + p_of * NT + t_of    # table row of each node
    col_of = rho                           # h column within core = rank

    # deg with self loop, [128, NT] per core, DEG_INF for phantoms
    deg_f = np.full((NC, 128, NT), DEG_INF, dtype=np.float32)
    deg_f[core, p_of, t_of] = (indeg + 1).astype(np.float32)

    # zero rows per window: a phantom row in [base, lo+WSPAN) (idx >= 0)
    phantom_rows = []
    for c in range(NC):
        for r in range(NPC_RAW, NPC):
            pp, tt = r % 128, r // 128
            phantom_rows.append(c * NPC + pp * NT + tt)
    phantom_rows = np.sort(np.array(phantom_rows, dtype=np.int64))
    zrow = []
    for w in range(NW):
        ok = phantom_rows[(phantom_rows >= WIN_BASE[w])
                          & (phantom_rows < WIN_LO[w] + WSPAN)]
        assert len(ok), f"no zero row for window {w}"
        zrow.append(int(ok[-1]))

    # ---- edge slot assignment ----
    # per (core, tile, partition): list of source rows, split across windows
    s_row = tid[src]
    d_core = core[dst]
    d_t = t_of[dst]
    d_p = p_of[dst]

    # windows containing each row: w is candidate iff LO[w] <= row < LO[w]+32K
    # row's candidate set = contiguous window range [wlo(row), whi(row)]
    lo_arr = np.array(WIN_LO, dtype=np.int64)
    whi = np.searchsorted(lo_arr, s_row, side="right") - 1          # last lo <= row
    wlo = np.searchsorted(lo_arr + WSPAN, s_row, side="right")      # first lo+32K > row
    assert (wlo <= whi).all()
    s_reg = wlo * NW + whi          # encode candidate range

    # order edges by (core, tile, partition) for grouping
    eorder = np.lexsort((s_row, d_p, d_t, d_core))
    sc, st, sp = d_core[eorder], d_t[eorder], d_p[eorder]
    sr, sg = s_row[eorder], s_reg[eorder]
    # group boundaries for each (c,t,p)
    key = (sc * NT + st) * 128 + sp
    grp_start = np.searchsorted(key, np.arange(NC * NT * 128))
    grp_end = np.concatenate([grp_start[1:], [len(key)]])

    # greedy per-(c,t,p) window assignment (deterministic; K = max load).
    # single-candidate rows first, then flex rows to the less-loaded window.
    # Vectorized over groups via per-edge sequential numpy is too slow in
    # pure python per edge; loop per (tile) with numpy inside.
    # spill-structured assignment (A-only->A, C-only->C, AB spills to B after
    # A's cap, BC to B after C's cap, ABC fills A leftover, C leftover, then B)
    lws, whs = wlo[eorder], whi[eorder]
    regid = np.full(len(sr), -1, dtype=np.int64)   # 0=A,1=AB,2=ABC,3=BC,4=C
    regid[(lws == 0) & (whs == 0)] = 0
    regid[(lws == 0) & (whs == 1)] = 1
    regid[(lws == 0) & (whs == 2)] = 2
    regid[(lws == 1) & (whs == 2)] = 3
    regid[(lws == 2) & (whs == 2)] = 4
    assert (regid >= 0).all()
    NGK = NC * NT * 128
    cnt_reg = np.zeros((NGK, 5), dtype=np.int64)
    np.add.at(cnt_reg, (key, regid), 1)
    nA, nAB, nABC, nBC, nC = (cnt_reg[:, i] for i in range(5))
    tile_key = (np.arange(NGK) // 128) % NT
    KA_t = np.zeros(NT, dtype=np.int64)
    np.maximum.at(KA_t, tile_key, nA)
    KC_t = np.zeros(NT, dtype=np.int64)
    np.maximum.at(KC_t, tile_key, nC)
    KA_g, KC_g = KA_t[tile_key], KC_t[tile_key]
    x_AB = np.clip(KA_g - nA, 0, nAB)
    x_BC = np.clip(KC_g - nC, 0, nBC)
    leftA = np.maximum(0, KA_g - nA - nAB)
    leftC = np.maximum(0, KC_g - nC - nBC)
    x_ABC_A = np.minimum(nABC, leftA)
    x_ABC_C = np.minimum(nABC - x_ABC_A, leftC)
    B_load = (nAB - x_AB) + (nBC - x_BC) + (nABC - x_ABC_A - x_ABC_C)
    KB_t = np.zeros(NT, dtype=np.int64)
    np.maximum.at(KB_t, tile_key, B_load)
    aw_all = np.empty(len(sr), dtype=np.int64)
    aw_all[regid == 0] = 0
    aw_all[regid == 4] = 2

    def _rank_split(mask, thresh_g, w_lo, w_hi):
        sub = np.nonzero(mask)[0]
        if not len(sub):
            return None
        gk = key[sub]
        uq, uq_start, inv = np.unique(gk, return_index=True, return_inverse=True)
        rank = np.arange(len(sub)) - uq_start[inv]
        aw_all[sub] = np.where(rank < thresh_g[gk], w_lo, w_hi)
        return sub, gk, rank

    _rank_split(regid == 1, x_AB, 0, 1)
    _rank_split(regid == 3, x_BC, 2, 1)
    # ABC: rank < x_ABC_A -> A; rank < x_ABC_A + x_ABC_C -> C; else B
    sub = np.nonzero(regid == 2)[0]
    if len(sub):
        gk = key[sub]
        uq, uq_start, inv = np.unique(gk, return_index=True, return_inverse=True)
        rank = np.arange(len(sub)) - uq_start[inv]
        aw = np.full(len(sub), 1, dtype=np.int64)
        aw[rank < x_ABC_A[gk]] = 0
        sel = (rank >= x_ABC_A[gk]) & (rank < (x_ABC_A + x_ABC_C)[gk])
        aw[sel] = 2
        aw_all[sub] = aw
    K_w_t = np.stack([KA_t, KB_t, KC_t])

    # chunking: group tiles until sum of real cols > CHUNK_COLS
    Ktot_t = K_w_t.sum(axis=0)
    chunks = []          # list of (t0, t1)
    t0 = 0
    acc = 0
    for t in range(NT):
        acc += int(Ktot_t[t])
        if acc >= CHUNK_COLS or t == NT - 1:
            chunks.append((t0, t + 1))
            t0 = t + 1
            acc = 0

    # slab layout per chunk: [w0 cols of tiles t0..t1-1][pad col]
    #                        [w1 cols...][pad col][w2 cols...][pad col]
    # build idx stream + per-chunk metadata
    idx_stream = []
    pos = 0
    chunk_meta = []      # (t0, t1, [(w, ncols, idxpos, slaboff)], slabcols)
    per_core_vals = [[] for _ in range(NC)]

    for (ct0, ct1) in chunks:
        gathers = []
        slaboff = 0
        for w in range(NW):
            ncols = _padlen(int(K_w_t[w, ct0:ct1].sum()))
            gathers.append((w, ncols, pos, slaboff))
            pos += ncols * 128
            slaboff += ncols
        chunk_meta.append((ct0, ct1, gathers, slaboff))
    total_pos = pos

    # fill per-core idx values
    vals = np.zeros((NC, total_pos), dtype=np.int64)
    for c in range(NC):
        v = vals[c]
        for (ct0, ct1, gathers, slabcols) in chunk_meta:
            for (w, ncols, ipos, _so) in gathers:
                v[ipos:ipos + ncols * 128] = zrow[w] - WIN_BASE[w]
        # place real edges using the global greedy assignment
        tile_of_chunk = {}
        for ci, (ct0, ct1, gathers, _sc) in enumerate(chunk_meta):
            for t in range(ct0, ct1):
                tile_of_chunk[t] = ci
        for t in range(NT):
            ci = tile_of_chunk[t]
            ct0, ct1, gathers, _sc2 = chunk_meta[ci]
            koff = [int(K_w_t[w, ct0:t].sum()) for w in range(NW)]
            for p in range(128):
                g = grp_start[(c * NT + t) * 128 + p]
                e = grp_end[(c * NT + t) * 128 + p]
                rows = sr[g:e]
                aw = aw_all[g:e]
                slotw = [0] * NW
                for i in range(len(rows)):
                    w = int(aw[i])
                    k = koff[w] + slotw[w]
                    slotw[w] += 1
                    gw = gathers[w]
                    j = gw[2] + _padpos(k) * 128 + p
                    v[j] = rows[i] - WIN_BASE[w]
        assert v.max() < 32768 and v.min() >= -32768
        per_core_vals[c] = v

    SK = sum(m[3] for m in chunk_meta)

    # ---- pooling ----
    cnt = np.bincount(batch, minlength=G).astype(np.int64)
    gstart = np.concatenate([[0], np.cumsum(cnt)])
    KG = int(max(1, int(np.ceil(cnt.max() / 128.0))))
    # per-core slot tables
    pool_idx = np.zeros((NC, NG_SLOT * KG * 128), dtype=np.int64)
    m0_idx = np.zeros((NC, 128), dtype=np.int64)
    route = np.zeros((NC, 128, G), dtype=np.float32)     # slot part -> graph
    noroute = np.full((NC, 1, G), NEG, dtype=np.float32)
    npad_sl = np.zeros((NC, 1, NG_SLOT), dtype=np.float32)
    local_row = p_of * NT + t_of                          # row within slice
    for c in range(NC):
        lo_id, hi_id = c * NPC_RAW, (c + 1) * NPC_RAW
        glist = [g for g in range(G)
                 if gstart[g] < hi_id and gstart[g + 1] > lo_id]
        assert len(glist) <= NG_SLOT, len(glist)
        for s, g in enumerate(glist):
            a = max(gstart[g], lo_id)
            b = min(gstart[g + 1], hi_id)
            mem = local_row[a:b]                          # local slice rows
            base = s * KG * 128
            pool_idx[c, base:base + KG * 128] = mem[0]
            pool_idx[c, base:base + len(mem)] = mem
            m0_idx[c, s] = mem[0]
            route[c, s, g] = 1.0
            noroute[c, 0, g] = 0.0
            npad_sl[c, 0, s] = KG * 128 - len(mem)
    cntinv = np.zeros((1, G), dtype=np.float32)
    nz = cnt > 0
    cntinv[0, nz] = 1.0 / cnt[nz]
    cntpos = nz.astype(np.float32).reshape(1, G)

    # permute idx (rank order -> id order), per core
    perm_idx = np.zeros((NC, NPC), dtype=np.int64)
    for c in range(NC):
        ids = np.arange(c * NPC_RAW, (c + 1) * NPC_RAW)
        perm_idx[c, :NPC_RAW] = local_row[ids]

    return dict(
        tid=tid, col_of=col_of, deg_f=deg_f, zrow=zrow,
        K_w_t=K_w_t, chunk_meta=chunk_meta, total_pos=total_pos, SK=SK,
        per_core_vals=per_core_vals,
        cnt=cnt, gstart=gstart, KG=KG, pool_idx=pool_idx, m0_idx=m0_idx,
        route=route, noroute=noroute, npad_sl=npad_sl, cntinv=cntinv,
        cntpos=cntpos, perm_idx=perm_idx, core=core,
    )


def _numpy_model(x, prep, Ws, gs, bes):
    """float32 mirror of the device algorithm (validation only)."""
    tid, col_of, core = prep["tid"], prep["col_of"], prep["core"]
    deg_f = prep["deg_f"]                                 # [NC,128,NT]
    dinv = np.sqrt(np.float32(1.0) / deg_f).astype(np.float32)
    chunk_meta = prep["chunk_meta"]
    K_w_t = prep["K_w_t"]
    per_core_vals = prep["per_core_vals"]

    # h feature-major per core: [NC, 64, NPC], column rho
    h_fm = np.zeros((NC, D, NPC), dtype=np.float32)
    h_fm[core, :, col_of] = x          # fancy: rows = nodes
    table = np.zeros((TR, D), dtype=np.float32)

    for l in range(3):
        W, g_, be = Ws[l], gs[l], bes[l]
        hhat = np.zeros((NC, 128, NT, D), dtype=np.float32)  # [p, t, f]
        for c in range(NC):
            t_fm = (W.T @ h_fm[c]).astype(np.float32)        # [64, NPC]
            for t in range(NT):
                blk = t_fm[:, t * 128:(t + 1) * 128].T       # [128p, 64]
                hhat[c, :, t, :] = blk * dinv[c, :, t][:, None]
            table[c * NPC:(c + 1) * NPC] = hhat[c].reshape(NPC, D)
        y = np.zeros((NC, 128, NT, D), dtype=np.float32)
        for c in range(NC):
            v = per_core_vals[c]
            for (ct0, ct1, gathers, slabcols) in chunk_meta:
                slab = np.zeros((128, slabcols, D), dtype=np.float32)
                for (w, ncols, ipos, so) in gathers:
                    base = WIN_BASE[w]
                    for col in range(ncols):
                        for p in range(128):
                            r = v[ipos + col * 128 + p] + base
                            slab[p, so + col] = table[r]
                for t in range(ct0, ct1):
                    msum = np.zeros((128, D), dtype=np.float32)
                    for w, (ww, ncols, ipos, so) in enumerate(gathers):
                        kr0 = int(K_w_t[w, ct0:t].sum())
                        kr1 = kr0 + int(K_w_t[w, t])
                        if kr1 > kr0:
                            k0 = so + _padpos(kr0)
                            k1 = so + _padpos(kr1 - 1) + 1
                            msum += slab[:, k0:k1].sum(axis=1)
                    acc = (msum + hhat[c, :, t, :]) * dinv[c, :, t][:, None]
                    y[c, :, t, :] = acc
        S = y.sum(axis=(0, 1, 2))
        Q = (y ** 2).sum(axis=(0, 1, 2))
        mean = (S / np.float32(N)).astype(np.float32)
        var = (Q / np.float32(N) - mean * mean).astype(np.float32)
        rstd = np.sqrt(np.float32(1.0) / (var + np.float32(EPS)))
        scale = (g_ * rstd).astype(np.float32)
        shift = (be - mean * scale).astype(np.float32)
        for c in range(NC):
            hn = y[c] * scale[None, None, :] + shift[None, None, :]
            if l < 2:
                hn = np.maximum(hn, 0)
            # back to feature-major, column rho = t*128+p
            for t in range(NT):
                h_fm[c][:, t * 128:(t + 1) * 128] = hn[:, t, :].T

    # pooling from slice3 (= hn of layer 3) via permute + chunks
    KG = prep["KG"]
    sumfull = np.zeros((D, G), dtype=np.float32)
    maxfull = np.full((D, G), NEG, dtype=np.float32)
    for c in range(NC):
        slice3 = np.zeros((NPC, D), dtype=np.float32)
        for t in range(NT):
            slice3[np.arange(128) * NT + t] = h_fm[c][:, t * 128:(t + 1) * 128].T
        pidx = prep["pool_idx"][c]
        slab = slice3[pidx]                       # [NG*KG*128, 64]
        sum36 = np.zeros((D, NG_SLOT), dtype=np.float32)
        max36 = np.full((D, NG_SLOT), NEG, dtype=np.float32)
        for s in range(NG_SLOT):
            seg = slab[s * KG * 128:(s + 1) * KG * 128]
            sum36[:, s] = seg.sum(axis=0)
            max36[:, s] = seg.max(axis=0)
        m0rows = slice3[prep["m0_idx"][c][:NG_SLOT]]          # [36, 64]
        sum36 = sum36 - prep["npad_sl"][c][0][None, :] * m0rows.T
        route = prep["route"][c][:NG_SLOT]                    # [36, G]
        sumfull += sum36 @ route
        mx = max36 @ route + prep["noroute"][c]               # [D, G]
        maxfull = np.maximum(maxfull, mx)
    mean = sumfull * prep["cntinv"]
    mx = maxfull * prep["cntpos"]
    out = (mean + mx).T.astype(np.float32)
    return out


def kernel(**inputs):
    x = np.asarray(inputs["x"], dtype=np.float32)
    prep = _preprocess(inputs["edge_index"], inputs["batch"])
    Ws = [np.asarray(inputs[f"W{i+1}"], dtype=np.float32) for i in range(3)]
    gs = [np.asarray(inputs[f"g{i+1}"], dtype=np.float32) for i in range(3)]
    bes = [np.asarray(inputs[f"be{i+1}"], dtype=np.float32) for i in range(3)]
    import os
    if os.environ.get("GCN_NUMPY_MODEL"):
        return _numpy_model(x, prep, Ws, gs, bes)
    return _run_device(x, prep, Ws, gs, bes)


_DEVICE_CACHE = {}
LAST_RESULT = None


def _run_device(x, prep, Ws, gs, bes):
    from concourse.bass_utils import run_bass_kernel_spmd
    import os

    key = (prep["total_pos"], prep["KG"],
           os.environ.get("GCN_REPS", "1"), os.environ.get("GCN_AGG_MODE", "full"),
           tuple(int(k) for k in prep["K_w_t"].flatten()))
    if key not in _DEVICE_CACHE:
        _DEVICE_CACHE[key] = _build_device(prep)
    nc = _DEVICE_CACHE[key]
    in_maps = _make_inmaps(x, prep, Ws, gs, bes)
    trace = bool(os.environ.get("GCN_TRACE"))
    res = run_bass_kernel_spmd(nc, in_maps, core_ids=list(range(NC)), trace=trace)
    global LAST_RESULT
    LAST_RESULT = res
    return np.asarray(res.results[0]["out"], dtype=np.float32)
